# revision 2
# baseline (speedup 1.0000x reference)
"""GQA kernel for Trainium2: B=2,T=2048,E=2048,G=4,QPG=4,D=128, causal + sinusoidal PE.

Sharding: one core per (batch, kv-group) pair = 2*4 = 8 cores.
Each core computes q/k/v projections for its group, attention for its 4 query
heads, and a partial output projection (its group's 512 columns of wo);
partials are summed on the host.

Layout strategy (per core):
  - all matmul streams are bf16 (halves DMA traffic vs f32, enables FWL
    ~54ns LDWEIGHTS everywhere); PSUM accumulation stays f32.
  - host passes x^T pre-blocked as [TB, 4, 128, 2048] bf16 so each DMA is a
    contiguous [128, 2048] tile holding 4 e-tiles; weights likewise.
  - xt streams on the sync DGE queue; weights on the scalar DGE queue
    (parallel rings) so the first matmul's operands land ASAP.
  - scores are computed transposed: S^T[tk, tq] = kt_tile.T @ qt, softmax'd
    without max subtraction (scores bounded, verified), exp'd into bf16 P^T
    tiles that feed the PV matmul directly as lhsT. Diagonal tiles only
    compute the valid suffix columns.
  - denominator comes free via a ones-column appended to V (N=129).
  - attention output [tq, d] is normalized via per-partition scale (bf16),
    then PE-transposed to [d, tq] to feed the wo matmul as lhsT.
"""
import sys

sys.path.insert(0, "/opt/trn_rl_repo")

import math
import numpy as np

B, T, E = 2, 2048, 2048
G, QPG, D = 4, 4, 128
NQ = QPG * D          # 512 q columns per group
NKV = 2 * D           # 256 kv columns per group
TT = T // 128         # 16 t-tiles
TB = T // 512         # 4 t-blocks
NE = E // 128         # 16 e-tiles
ISD = 1.0 / math.sqrt(D)

_compiled = None


def _build():
    from concourse import bacc, tile, mybir

    f32 = mybir.dt.float32
    bf16 = mybir.dt.bfloat16
    ADD = mybir.AluOpType.add
    MULT = mybir.AluOpType.mult
    EXP = mybir.ActivationFunctionType.Exp
    IDENT = mybir.ActivationFunctionType.Identity

    nc = bacc.Bacc("TRN2", target_bir_lowering=False, debug=False, num_devices=8)

    xt_d = nc.dram_tensor("xt", [TB, 4, 128, 4 * 512], bf16, kind="ExternalInput")   # x^T blocked
    wq_d = nc.dram_tensor("wq", [4, 128, 4 * 512], bf16, kind="ExternalInput")       # group slice blocked
    wkv_d = nc.dram_tensor("wkv", [4, 128, 4 * 256], bf16, kind="ExternalInput")     # group slice blocked
    wo_d = nc.dram_tensor("wo", [4, 128, E], bf16, kind="ExternalInput")             # group slice
    pet_d = nc.dram_tensor("pet", [D, T], f32, kind="ExternalInput")                 # pe^T
    bq_d = nc.dram_tensor("bq", [D, QPG], f32, kind="ExternalInput")                 # col h
    bk_d = nc.dram_tensor("bk", [D, 1], f32, kind="ExternalInput")
    bv_d = nc.dram_tensor("bv", [D, 1], f32, kind="ExternalInput")
    msk_d = nc.dram_tensor("msk", [4, 128, 512], bf16, kind="ExternalInput")
    idb_d = nc.dram_tensor("idb", [128, 128], bf16, kind="ExternalInput")
    ones_d = nc.dram_tensor("ones1", [128, 1], bf16, kind="ExternalInput")
    out_d = nc.dram_tensor("out", [T, E], f32, kind="ExternalOutput")

    with tile.TileContext(nc) as tc:
        with tc.tile_pool(name="persist", bufs=1) as pp:
            qt = [pp.tile([128, T], bf16, name=f"qt{h}", tag=f"qt{h}") for h in range(QPG)]
            kt = pp.tile([128, T], bf16)
            vext = [pp.tile([128, 132], bf16, name=f"vx{i}", tag=f"vx{i}") for i in range(TT)]
            at = [pp.tile([128, T], bf16, name=f"at{h}", tag=f"at{h}") for h in range(QPG)]
            wo_sb = [pp.tile([128, E], bf16, name=f"wo{h}", tag=f"wo{h}") for h in range(QPG)]

            # ---- persistent constants: gpsimd (sw DGE) queue, after nothing
            # critical; weights go on the scalar DGE ring so they stream in
            # parallel with the sync ring's xt tiles. ----
            pet = pp.tile([D, T], f32)
            bq = pp.tile([D, QPG], f32)
            bk = pp.tile([D, 1], f32)
            bv = pp.tile([D, 1], f32)
            idb = pp.tile([128, 128], bf16)
            ones1 = pp.tile([128, 1], bf16)
            msk = [pp.tile([128, 512], bf16, name=f"msk{j}", tag=f"msk{j}") for j in range(4)]

            # ---- phase 1: projections ----
            with (
                tc.tile_pool(name="p1", bufs=1) as p1,
                tc.tile_pool(name="p1x", bufs=8) as p1x,
                tc.tile_pool(name="psA", bufs=1, space="PSUM") as psA,
                tc.tile_pool(name="ps1b", bufs=2, space="PSUM") as ps1b,
            ):
                wq_sb = [p1.tile([128, 4 * 512], bf16, name=f"wq{a}", tag=f"wq{a}") for a in range(4)]
                wkv_sb = [p1.tile([128, 4 * 256], bf16, name=f"wkv{a}", tag=f"wkv{a}") for a in range(4)]
                xt0 = [p1x.tile([128, 4 * 512], bf16, name=f"xt0_{a}", tag="xt") for a in range(4)]
                # first matmul needs wq_sb[0] + xt0[0]: first trigger on each ring
                nc.scalar.dma_start(wq_sb[0][:], wq_d[0])
                nc.sync.dma_start(xt0[0][:], xt_d[0, 0])
                nc.scalar.dma_start(wkv_sb[0][:], wkv_d[0])
                nc.sync.dma_start(xt0[1][:], xt_d[0, 1])
                for a in range(1, 4):
                    nc.scalar.dma_start(wq_sb[a][:], wq_d[a])
                    nc.scalar.dma_start(wkv_sb[a][:], wkv_d[a])
                for a in range(2, 4):
                    nc.sync.dma_start(xt0[a][:], xt_d[0, a])
                # constants on gpsimd (parallel, lower priority)
                nc.gpsimd.dma_start(pet[:], pet_d[:])
                nc.gpsimd.dma_start(bq[:], bq_d[:])
                nc.gpsimd.dma_start(bk[:], bk_d[:])
                nc.gpsimd.dma_start(bv[:], bv_d[:])
                nc.gpsimd.dma_start(idb[:], idb_d[:])
                nc.gpsimd.dma_start(ones1[:], ones_d[:])
                for j in range(4):
                    nc.gpsimd.dma_start(msk[j][:], msk_d[j])

                for tb in range(TB):
                    ts = slice(tb * 512, (tb + 1) * 512)
                    qt_ps = psA.tile([128, 4 * 512], f32, name="qt_ps", tag="qt_ps")
                    kt_ps = psA.tile([128, 512], f32, name="kt_ps", tag="kt_ps")
                    vt_ps = psA.tile([128, 512], f32, name="vt_ps", tag="vt_ps")
                    for e in range(NE):
                        a, b = divmod(e, 4)
                        if tb == 0:
                            xt_t = xt0[a]
                        elif b == 0:
                            xt_t = p1x.tile([128, 4 * 512], bf16, name="xt", tag="xt")
                            nc.sync.dma_start(xt_t[:], xt_d[tb, a])
                        xs = slice(b * 512, (b + 1) * 512)
                        st = e == 0
                        sp = e == NE - 1
                        for h in range(QPG):
                            nc.tensor.matmul(
                                qt_ps[:, h * 512:(h + 1) * 512],
                                wq_sb[a][:, b * 512 + h * 128:b * 512 + (h + 1) * 128],
                                xt_t[:, xs], start=st, stop=sp,
                            )
                        nc.tensor.matmul(kt_ps[:], wkv_sb[a][:, b * 256:b * 256 + 128], xt_t[:, xs], start=st, stop=sp)
                        nc.tensor.matmul(vt_ps[:], wkv_sb[a][:, b * 256 + 128:b * 256 + 256], xt_t[:, xs], start=st, stop=sp)
                    # drain: bias (in-place on psum) then += pe^T -> sbuf bf16
                    for h in range(QPG):
                        sl = qt_ps[:, h * 512:(h + 1) * 512]
                        nc.vector.tensor_tensor(sl, sl, bq[:, h:h + 1].to_broadcast([128, 512]), ADD)
                        nc.vector.tensor_tensor(qt[h][:, ts], sl, pet[:, ts], ADD)
                    nc.vector.tensor_tensor(kt_ps[:], kt_ps[:], bk[:].to_broadcast([128, 512]), ADD)
                    nc.vector.tensor_tensor(kt[:, ts], kt_ps[:], pet[:, ts], ADD)
                    # v: bias then cast to bf16, then transpose each 128-tile
                    vtb = p1.tile([128, 512], bf16, name="vtb", tag="vtb")
                    nc.scalar.activation(vtb[:], vt_ps[:], IDENT, bias=bv[:], scale=1.0)
                    for i in range(4):
                        ti = tb * 4 + i
                        vtp = ps1b.tile([128, 128], bf16, name="vtp", tag="vtp")
                        nc.tensor.transpose(vtp[:], vtb[:, i * 128:(i + 1) * 128], idb[:])
                        nc.vector.tensor_copy(vext[ti][:, 0:128], vtp[:])
                        nc.vector.tensor_copy(vext[ti][:, 128:129], ones1[:])

            for h in range(QPG):
                nc.gpsimd.dma_start(wo_sb[h][:], wo_d[h])

            # ---- phase 2+3: attention fused with output projection ----
            # Emission is software-pipelined. During a head's S^T score stretch
            # the in-order PE is paced by ACT's exp; wo-projection matmuls fill
            # those gaps at ~1 unit per S^T tile. PV runs as pure bf16 streaks
            # (FWL keeps LDWEIGHTS at ~54ns) with the DVE epilogues batched
            # after each streak.
            with (
                tc.tile_pool(name="p2", bufs=17) as p2,
                tc.tile_pool(name="p2s", bufs=8) as p2s,
                tc.tile_pool(name="p3", bufs=3) as p3,
                tc.tile_pool(name="ps2", bufs=2, space="PSUM") as ps2,
            ):
                from collections import deque
                filler = deque()

                def drain(n):
                    for _ in range(n):
                        if not filler:
                            return
                        filler.popleft()()

                def wo_units(qb):
                    units = []
                    for jj in range(4):
                        ti = qb * 4 + jj
                        state = {}

                        def alloc(state=state):
                            state["o_sb"] = p3.tile([128, E], f32, name="osb", tag="osb")
                        units.append(alloc)
                        for eo in range(4):
                            def mmA(state=state, ti=ti, eo=eo):
                                w_ps = ps2.tile([128, 512], f32, name="w_ps", tag="mix", bufs=2)
                                state["w"] = w_ps
                                for h in range(2):
                                    nc.tensor.matmul(
                                        w_ps[:], at[h][:, ti * 128:(ti + 1) * 128],
                                        wo_sb[h][:, eo * 512:(eo + 1) * 512],
                                        start=(h == 0), stop=False,
                                    )

                            def mmB(state=state, ti=ti, eo=eo):
                                w_ps = state["w"]
                                for h in range(2, 4):
                                    nc.tensor.matmul(
                                        w_ps[:], at[h][:, ti * 128:(ti + 1) * 128],
                                        wo_sb[h][:, eo * 512:(eo + 1) * 512],
                                        start=False, stop=(h == 3),
                                    )
                                nc.vector.tensor_copy(state["o_sb"][:, eo * 512:(eo + 1) * 512], w_ps[:])
                            units.append(mmA)
                            units.append(mmB)

                        def store(state=state, ti=ti):
                            nc.sync.dma_start(out_d[ti * 128:(ti + 1) * 128, :], state["o_sb"][:])
                        units.append(store)
                    return units

                def make_pv_streak(h, qb, pt, and_then=None):
                    def emit():
                        # pure bf16 PV streak: all four tq sub-tiles back to back
                        o_list = []
                        for j in range(4):
                            tt = 4 * qb + j
                            o_ps = ps2.tile([128, 129], f32, name="o_ps", tag="o_ps", bufs=4)
                            o_list.append(o_ps)
                            for tk in range(tt + 1):
                                nc.tensor.matmul(
                                    o_ps[:], pt[tk][:, j * 128:(j + 1) * 128],
                                    vext[tk][:, 0:129],
                                    start=(tk == 0), stop=(tk == tt),
                                )
                        for j in range(4):
                            tt = 4 * qb + j
                            o_ps = o_list[j]
                            r_sb = p2s.tile([128, 1], f32, name="r", tag="r")
                            nc.vector.reciprocal(r_sb[:], o_ps[:, 128:129])
                            a_sb = p2s.tile([128, 128], bf16, name="a", tag="a")
                            nc.vector.tensor_tensor(
                                a_sb[:], o_ps[:, 0:128], r_sb[:].to_broadcast([128, 128]), MULT,
                            )
                            at_ps = ps2.tile([128, 128], bf16, name="at_ps", tag="mix", bufs=2)
                            nc.tensor.transpose(at_ps[:], a_sb[:], idb[:])
                            nc.vector.tensor_copy(at[h][:, tt * 128:(tt + 1) * 128], at_ps[:])
                        if and_then is not None:
                            and_then()
                    return emit

                pending_pv = None
                for qb in range(TB):
                    nkt = 4 * qb + 4
                    for h in range(QPG):
                        pt = []
                        for tk in range(nkt):
                            j = tk - 4 * qb
                            off = max(j, 0) * 128
                            w = 512 - off
                            qcols = slice(qb * 512 + off, (qb + 1) * 512)
                            s_ps = ps2.tile([128, 512], f32, name="s_ps", tag="s_ps")
                            nc.tensor.matmul(
                                s_ps[:, 0:w], kt[:, tk * 128:(tk + 1) * 128], qt[h][:, qcols],
                                start=True, stop=True,
                            )
                            p_t = p2.tile([128, 512], bf16, name="pt", tag="pt")
                            nc.scalar.activation(p_t[:, off:], s_ps[:, 0:w], EXP, scale=ISD)
                            if j >= 0:
                                nc.vector.tensor_tensor(p_t[:, off:], p_t[:, off:], msk[j][:, off:], MULT)
                            pt.append(p_t)
                            if tk == 1 and pending_pv is not None:
                                pending_pv()
                                pending_pv = None
                            else:
                                drain(1)
                        cb = None
                        if h == QPG - 1:
                            def cb(qb=qb):
                                filler.extend(wo_units(qb))
                        pending_pv = make_pv_streak(h, qb, pt, and_then=cb)
                if pending_pv is not None:
                    pending_pv()
                drain(len(filler) + 1)

    nc.compile()
    return nc


def _get_compiled():
    global _compiled
    if _compiled is None:
        _compiled = _build()
    return _compiled


def _host_inputs(x, wq, bq, wkv, bkv, wo):
    import ml_dtypes

    bf = ml_dtypes.bfloat16

    pos = np.arange(T, dtype=np.float32)[:, None]
    i = np.arange(0, D, 2, dtype=np.float32)
    inv = np.exp(-(np.log(10000.0) * i / D))
    ang = pos * inv
    pe = np.zeros((T, D), np.float32)
    pe[:, 0::2] = np.sin(ang)
    pe[:, 1::2] = np.cos(ang)
    pet = np.ascontiguousarray(pe.T)

    # causal masks for the 4 diagonal tiles of a 512-wide tq block:
    # mask_j[p, c] = 1 if c >= 128*j + p
    c = np.arange(512)[None, :]
    p = np.arange(128)[:, None]
    msk = np.stack([(c >= 128 * j + p) for j in range(4)]).astype(bf)

    idb = np.eye(128, dtype=bf)
    ones1 = np.ones((128, 1), dtype=bf)

    # x^T blocked: [TB, a, p, b*512+c] = xT[(4a+b)*128+p, tb*512+c]
    xts = []
    for b_ in range(B):
        xT = np.ascontiguousarray(x[b_].T).astype(bf)          # [E, T]
        xb = xT.reshape(4, 4, 128, 4, 512).transpose(3, 0, 2, 1, 4).reshape(TB, 4, 128, 4 * 512)
        xts.append(np.ascontiguousarray(xb))

    in_maps = []
    for core in range(8):
        b_, g = divmod(core, G)
        wqg = wq[:, g * NQ:(g + 1) * NQ].astype(bf)            # [E, 512]
        wqb = wqg.reshape(4, 4, 128, 512).transpose(0, 2, 1, 3).reshape(4, 128, 4 * 512)
        wkvg = wkv[:, g * NKV:(g + 1) * NKV].astype(bf)        # [E, 256]
        wkvb = wkvg.reshape(4, 4, 128, 256).transpose(0, 2, 1, 3).reshape(4, 128, 4 * 256)
        wog = wo[g * NQ:(g + 1) * NQ, :].astype(bf).reshape(4, 128, E)
        in_maps.append({
            "xt": xts[b_],
            "wq": np.ascontiguousarray(wqb),
            "wkv": np.ascontiguousarray(wkvb),
            "wo": np.ascontiguousarray(wog),
            "pet": pet,
            "bq": np.ascontiguousarray(bq[g * NQ:(g + 1) * NQ].reshape(QPG, D).T),
            "bk": np.ascontiguousarray(bkv[g * NKV:g * NKV + D].reshape(D, 1)),
            "bv": np.ascontiguousarray(bkv[g * NKV + D:(g + 1) * NKV].reshape(D, 1)),
            "msk": msk,
            "idb": idb,
            "ones1": ones1,
        })
    return in_maps


def run(x, wq, bq, wkv, bkv, wo, trace=False):
    from concourse.bass_utils import run_bass_kernel_spmd

    nc = _get_compiled()
    in_maps = _host_inputs(
        np.asarray(x, np.float32), np.asarray(wq, np.float32),
        np.asarray(bq, np.float32), np.asarray(wkv, np.float32),
        np.asarray(bkv, np.float32), np.asarray(wo, np.float32),
    )
    res = run_bass_kernel_spmd(nc, in_maps, core_ids=list(range(8)), trace=trace)
    out = np.zeros((B, T, E), np.float32)
    for core in range(8):
        b_ = core // G
        out[b_] += res.results[core]["out"]
    return out, res


def kernel(x, wq, bq, wkv, bkv, wo):
    out, _ = run(x, wq, bq, wkv, bkv, wo, trace=False)
    return out
